# revision 2
# baseline (speedup 1.0000x reference)
"""HSIC loss kernel for Trainium2 (8 NeuronCores, Bass/Tile).

hsic = sum(L * HKH) / (m-1)^2
     = (S_LK - (2/m) kv.lv + sK*sL/m^2) / (m-1)^2
where K = exp(-dx), L = exp(-dy) (Gaussian kernels, sigma=1),
kv/lv = row sums of K/L, sK/sL = total sums, S_LK = sum(K*L).

Sharding: row-blocks of m/8 samples per core. Each core computes, for its
rows i and all columns j:
  accK[i-tile, j-chunk-group] = sum_j exp(2 x_i.x_j - sqx_i - sqx_j)
  accL likewise for y, accP likewise for the elementwise product K*L.
Host combines the tiny per-core partial sums in float64.

On-chip per 128x1024 tile (j processed in 1024-wide groups = 2 PSUM banks):
  PE  : G_K = x_i.x_j - sqx_j/2   (bf16 matmul + rank-2 hi/lo -sqx/2 rows)
        G_L = y_i.y_j - sqy_j/2   (single K=18 augmented matmul)
  ACT : K = exp(2*G_K + bias_i),  bias_i = -sqx_i  (per-partition, fp32),
        with accum_out giving the row sums for free. Same for L.
  DVE : P = K*L with accum_out giving sum(K*L) row partials.

All matmuls are bf16; exactness of the Gram diagonal (the only entries that
matter at fp32 scale) is preserved by computing the squared norms on host
from the *bf16-rounded* inputs and carrying -sq/2 as a hi/lo bf16 pair.
"""

import numpy as np
import ml_dtypes

M = 8192
DX = 128
DY = 16
NCORES = 8
R = M // NCORES          # rows per core = 1024
IT = R // 128            # i-tiles per core = 8
JW = 1024                # j-group width (2 PSUM banks)
JG = M // JW             # j-groups = 8
NACC = IT * JG           # accumulator columns = 64

_CACHE = {}


def _build_program():
    """Build + compile the SPMD Bass program (identical for all cores)."""
    from contextlib import ExitStack

    import concourse.bacc as bacc
    import concourse.tile as tile
    from concourse import mybir

    nc = bacc.Bacc(
        "TRN2",
        target_bir_lowering=False,
        debug=False,
        num_devices=NCORES,
    )
    bf16 = mybir.dt.bfloat16
    f32 = mybir.dt.float32

    # Per-core DRAM inputs
    xtb_d = nc.dram_tensor("xtb", [DX, M], bf16, kind="ExternalInput").ap()
    xti_d = nc.dram_tensor("xti", [DX, R], bf16, kind="ExternalInput").ap()
    xsq_d = nc.dram_tensor("xsq", [2, M], bf16, kind="ExternalInput").ap()
    ytb_d = nc.dram_tensor("ytb", [DY + 2, M], bf16, kind="ExternalInput").ap()
    yti_d = nc.dram_tensor("yti", [DY + 2, R], bf16, kind="ExternalInput").ap()
    bx_d = nc.dram_tensor("bx", [128, IT], f32, kind="ExternalInput").ap()
    by_d = nc.dram_tensor("by", [128, IT], f32, kind="ExternalInput").ap()

    accK_d = nc.dram_tensor("accK", [128, NACC], f32, kind="ExternalOutput").ap()
    accL_d = nc.dram_tensor("accL", [128, NACC], f32, kind="ExternalOutput").ap()
    accP_d = nc.dram_tensor("accP", [128, NACC], f32, kind="ExternalOutput").ap()

    with tile.TileContext(nc) as tc, ExitStack() as ctx:
        singles = ctx.enter_context(tc.tile_pool(name="singles", bufs=1))
        work = ctx.enter_context(tc.tile_pool(name="work", bufs=3))
        psum = ctx.enter_context(tc.tile_pool(name="psum", bufs=2, space="PSUM"))

        # Resident SBUF tensors
        xtb = singles.tile([DX, M], bf16)
        xti = singles.tile([DX, R], bf16)
        xsq = singles.tile([2, M], bf16)
        ytb = singles.tile([DY + 2, M], bf16)
        yti = singles.tile([DY + 2, R], bf16)
        bx = singles.tile([128, IT], f32)
        by = singles.tile([128, IT], f32)
        ones2 = singles.tile([2, 128], bf16)
        accK = singles.tile([128, NACC], f32)
        accL = singles.tile([128, NACC], f32)
        accP = singles.tile([128, NACC], f32)

        nc.sync.dma_start(out=xti, in_=xti_d)
        nc.sync.dma_start(out=yti, in_=yti_d)
        nc.sync.dma_start(out=xsq, in_=xsq_d)
        nc.sync.dma_start(out=bx, in_=bx_d)
        nc.sync.dma_start(out=by, in_=by_d)
        nc.sync.dma_start(out=ytb, in_=ytb_d)
        nc.sync.dma_start(out=xtb, in_=xtb_d)
        nc.vector.memset(ones2, 1.0)

        exp = mybir.ActivationFunctionType.Exp
        mult = mybir.AluOpType.mult

        for it in range(IT):
            for jg in range(JG):
                col = it * JG + jg
                gk = psum.tile([128, JW], f32, tag="gk")
                gl = psum.tile([128, JW], f32, tag="gl")
                for h in range(JW // 512):
                    j0 = jg * JW + h * 512
                    # G_K[:, h*512:...] = x_i.x_j - sqx_j/2
                    nc.tensor.matmul(
                        gk[:, h * 512 : (h + 1) * 512],
                        ones2,
                        xsq[:, j0 : j0 + 512],
                        start=True,
                        stop=False,
                    )
                    nc.tensor.matmul(
                        gk[:, h * 512 : (h + 1) * 512],
                        xti[:, it * 128 : (it + 1) * 128],
                        xtb[:, j0 : j0 + 512],
                        start=False,
                        stop=True,
                    )
                    # G_L[:, h*512:...] = y_i.y_j - sqy_j/2 (augmented rows)
                    nc.tensor.matmul(
                        gl[:, h * 512 : (h + 1) * 512],
                        yti[:, it * 128 : (it + 1) * 128],
                        ytb[:, j0 : j0 + 512],
                        start=True,
                        stop=True,
                    )
                ksb = work.tile([128, JW], bf16, tag="ksb")
                lsb = work.tile([128, JW], bf16, tag="lsb")
                psb = work.tile([128, JW], bf16, tag="psb")
                nc.scalar.activation(
                    out=ksb,
                    in_=gk,
                    func=exp,
                    bias=bx[:, it : it + 1],
                    scale=2.0,
                    accum_out=accK[:, col : col + 1],
                )
                nc.scalar.activation(
                    out=lsb,
                    in_=gl,
                    func=exp,
                    bias=by[:, it : it + 1],
                    scale=2.0,
                    accum_out=accL[:, col : col + 1],
                )
                nc.vector.scalar_tensor_tensor(
                    out=psb,
                    in0=ksb,
                    scalar=1.0,
                    in1=lsb,
                    op0=mult,
                    op1=mult,
                    accum_out=accP[:, col : col + 1],
                )

        nc.sync.dma_start(out=accK_d, in_=accK)
        nc.sync.dma_start(out=accL_d, in_=accL)
        nc.sync.dma_start(out=accP_d, in_=accP)

    nc.compile()
    return nc


def _split_hi_lo(a):
    """Split float64 vector into hi+lo bf16 pair summing to ~a."""
    h = a.astype(ml_dtypes.bfloat16)
    l = (a - h.astype(np.float64)).astype(ml_dtypes.bfloat16)
    return h, l


def _prepare_in_maps(x, y):
    xb = x.astype(ml_dtypes.bfloat16)
    yb = y.astype(ml_dtypes.bfloat16)
    x64 = xb.astype(np.float64)
    y64 = yb.astype(np.float64)
    sqx = (x64 * x64).sum(axis=1)  # [M]
    sqy = (y64 * y64).sum(axis=1)

    xsqh, xsql = _split_hi_lo(-0.5 * sqx)
    ysqh, ysql = _split_hi_lo(-0.5 * sqy)

    xtb = np.ascontiguousarray(xb.T)                      # [DX, M]
    ytb = np.concatenate([np.ascontiguousarray(yb.T), ysqh[None], ysql[None]], axis=0)
    xsq = np.stack([xsqh, xsql], axis=0)                  # [2, M]

    in_maps = []
    for c in range(NCORES):
        r0 = c * R
        yti = np.concatenate(
            [
                np.ascontiguousarray(yb[r0 : r0 + R].T),
                np.ones((2, R), dtype=ml_dtypes.bfloat16),
            ],
            axis=0,
        )
        bx = np.ascontiguousarray(
            -sqx[r0 : r0 + R].reshape(IT, 128).T.astype(np.float32)
        )
        by = np.ascontiguousarray(
            -sqy[r0 : r0 + R].reshape(IT, 128).T.astype(np.float32)
        )
        in_maps.append(
            {
                "xtb": xtb,
                "xti": np.ascontiguousarray(xb[r0 : r0 + R].T),
                "xsq": xsq,
                "ytb": ytb,
                "yti": yti,
                "bx": bx,
                "by": by,
            }
        )
    return in_maps


def _combine(results):
    """Host-side reduction of per-core partial sums -> hsic scalar."""
    m = float(M)
    kv = np.zeros(M, dtype=np.float64)
    lv = np.zeros(M, dtype=np.float64)
    s_lk = 0.0
    for c, res in enumerate(results):
        aK = res["accK"].astype(np.float64)  # [128, IT*JG]
        aL = res["accL"].astype(np.float64)
        aP = res["accP"].astype(np.float64)
        for it in range(IT):
            rows = slice(c * R + it * 128, c * R + (it + 1) * 128)
            kv[rows] = aK[:, it * JG : (it + 1) * JG].sum(axis=1)
            lv[rows] = aL[:, it * JG : (it + 1) * JG].sum(axis=1)
        s_lk += aP.sum()
    sK = kv.sum()
    sL = lv.sum()
    hsic = (s_lk - (2.0 / m) * np.dot(kv, lv) + sK * sL / (m * m)) / (m - 1.0) ** 2
    return np.float32(hsic)


def get_program():
    if "nc" not in _CACHE:
        _CACHE["nc"] = _build_program()
    return _CACHE["nc"]


def run_on_cores(in_maps):
    from concourse.bass_utils import run_bass_kernel_spmd

    nc = get_program()
    res = run_bass_kernel_spmd(nc, in_maps, core_ids=list(range(NCORES)))
    return res.results


def kernel(x, y):
    x = np.asarray(x)
    y = np.asarray(y)
    assert x.shape == (M, DX) and y.shape == (M, DY), (x.shape, y.shape)
    in_maps = _prepare_in_maps(x, y)
    results = run_on_cores(in_maps)
    return _combine(results)
